# revision 17
# baseline (speedup 1.0000x reference)
"""LengthRegulator (TTS duration-based token repeat) on 8 Trainium2 cores.

Reference semantics (per batch row b):
    ends = cumsum(durations[b])                      # [S]
    idx[t] = searchsorted(ends, t, side="right")     # first j with t < ends[j]
    out[b, t, :] = enc[b, min(idx[t], S-1), :] if t < ends[-1] else 0

Device algorithm (per core = 2 batch rows), gather formulation built on the
HW-verified indirect-DMA shapes (one offset per partition) plus dma_gather:

  idx[t] = #{j: ends[j] <= t}; for t >= total this is exactly S, so gathering
  from a host-staged table enc_ext = [enc; zeros] (S+1 rows) yields the
  truncated/zero-padded output with no masking.

  idx is computed by scattering markers into a zeroed DRAM array M[T]:
  for each token j that is the last of its equal-ends run (dur[j+1] > 0),
  M[ends[j]] = j+1 (offsets >= T dropped by the bounds check).  Then
  idx[t] = running-max of M over [0, t], evaluated as a per-partition
  free-dim scan on a [16, 256] layout (t = 256 q + c) combined with a
  cross-partition carry[q] = #{j: ends[j] < 256 q} from one matmul.
  idx is stored int16 to DRAM and reloaded in dma_gather's round-robin
  index layout ([16, 256], index i at partition i%16, col i//16); one
  dma_gather per row pulls all 4096 frames into SBUF, one DMA stores them.
"""

from contextlib import ExitStack

import numpy as np

import concourse.bacc as bacc
import concourse.bass as bass
import concourse.mybir as mybir
import concourse.tile as tile
from concourse.alu_op_type import AluOpType
from concourse.bass import AP, IndirectOffsetOnAxis

B, S, H = 16, 512, 384
T = 4096  # max_length
N_CORES = 8
RPC = B // N_CORES  # batch rows per core
P = 128
C = S // P  # tokens per partition (4)
Q = 16  # scan partitions; t = 256*q + c
TQ = T // Q  # 256
BIG = 1 << 20  # offset marker for dropped scatter elements

_F32 = mybir.dt.float32
_I32 = mybir.dt.int32
_I16 = mybir.dt.int16


def _view(t, pairs):
    """SBUF tile view with custom free-dim [step, count] pairs (step 0 = repeat)."""
    a = t[:]
    return AP(a.tensor, a.offset, [list(a.ap[0])] + [list(p) for p in pairs])


def build_program() -> bass.Bass:
    nc = bacc.Bacc()
    # enc_ext: encoder rows + one zero row (host-staged), gather table.
    # One tensor per row: the indirect-DMA side must be an offset-0 AP.
    encs = [
        nc.dram_tensor(f"enc{b}", [S + 1, H], _F32, kind="ExternalInput")
        for b in range(RPC)
    ]
    # dur: int32 durations + trailing 1 (host-staged) so dur[j+1] is always
    # readable and token S-1 is always "last of its run".
    dur = nc.dram_tensor("dur", [RPC, S + 1], _I32, kind="ExternalInput")
    ys = [
        nc.dram_tensor(f"y{b}", [T, H], _F32, kind="ExternalOutput") for b in range(RPC)
    ]
    mds = [nc.dram_tensor(f"m{b}", [T], _I32) for b in range(RPC)]
    ids = [nc.dram_tensor(f"i{b}", [T], _I16) for b in range(RPC)]

    with tile.TileContext(nc) as tc, ExitStack() as ctx:
        const = ctx.enter_context(tc.tile_pool(name="const", bufs=1))
        work = ctx.enter_context(tc.tile_pool(name="work", bufs=2))
        psum = ctx.enter_context(tc.tile_pool(name="psum", bufs=2, space="PSUM"))

        ones_pp = const.tile([P, P], _F32)
        nc.vector.memset(ones_pp[:], 1.0)
        ones_t = const.tile([P, 1], _F32)
        nc.vector.memset(ones_t[:], 1.0)
        zero_i = const.tile([P, T // P], _I32)
        nc.vector.memset(zero_i[:], 0)
        # ltri_T[k, p] = 1 iff k < p (built on gpsimd, copied through DVE so
        # the PE matmul depends on a single engine).
        ltri_raw = const.tile([P, P], _F32)
        nc.gpsimd.affine_select(
            out=ltri_raw[:],
            in_=ones_pp[:],
            pattern=[[1, P]],
            compare_op=AluOpType.is_gt,
            fill=0.0,
            base=0,
            channel_multiplier=-1,
        )
        ltri_T = const.tile([P, P], _F32)
        nc.vector.tensor_copy(ltri_T[:], ltri_raw[:])

        for b in range(RPC):
            # --- cumsum of durations -> inclusive ends [128, 4] (j = 4p+c)
            dur_sb = work.tile([P, C], _I32)
            nc.sync.dma_start(
                dur_sb[:], dur[b][0:S].rearrange("(p c) -> p c", p=P)
            )
            dur_nx = work.tile([P, C], _I32)
            nc.sync.dma_start(
                dur_nx[:],
                AP(dur[b].tensor, dur[b].offset + 1, [[C, P], [1, C]]),
            )
            dur_f = work.tile([P, C], _F32)
            nc.vector.tensor_copy(dur_f[:], dur_sb[:])
            incl = work.tile([P, C], _F32)
            nc.vector.tensor_tensor_scan(
                out=incl[:],
                data0=dur_f[:],
                data1=dur_f[:],
                initial=0.0,
                op0=AluOpType.add,
                op1=AluOpType.bypass,
            )
            o_ps = psum.tile([P, 1], _F32)
            nc.tensor.matmul(
                out=o_ps[:], lhsT=ltri_T[:], rhs=incl[:, C - 1 : C], start=True, stop=True
            )
            ends_f = work.tile([P, C], _F32)
            nc.vector.tensor_tensor(
                out=ends_f[:],
                in0=incl[:],
                in1=o_ps[:].to_broadcast([P, C]),
                op=AluOpType.add,
            )
            ends_i = work.tile([P, C], _I32)
            nc.vector.tensor_copy(ends_i[:], ends_f[:])

            # --- markers: M[ends[j]] = j+1 for last-of-run tokens
            jval = work.tile([P, C], _I32)
            nc.gpsimd.iota(jval[:], pattern=[[1, C]], base=1, channel_multiplier=C)
            inv = work.tile([P, C], _I32)
            nc.vector.tensor_scalar(
                out=inv[:], in0=dur_nx[:], scalar1=0, scalar2=None, op0=AluOpType.is_le
            )
            moff = work.tile([P, C], _I32)
            nc.vector.scalar_tensor_tensor(
                out=moff[:],
                in0=inv[:],
                scalar=BIG,
                in1=ends_i[:],
                op0=AluOpType.mult,
                op1=AluOpType.add,
            )
            # zero M, then scatter markers (one offset per partition per instr)
            nc.sync.dma_start(mds[b].rearrange("(p c) -> p c", p=P), zero_i[:])
            ma = mds[b][:]
            ma2 = AP(ma.tensor, ma.offset, [[1, T], [1, 1]])
            for c in range(C):
                nc.gpsimd.indirect_dma_start(
                    out=ma2,
                    out_offset=IndirectOffsetOnAxis(ap=moff[:, c : c + 1], axis=0),
                    in_=jval[:, c : c + 1],
                    in_offset=None,
                    bounds_check=T - 1,
                    oob_is_err=False,
                )

            # --- idx[t] = max(running-max of M within partition, carry[q])
            m_sb = work.tile([P, T // P], _I32)
            nc.sync.dma_start(m_sb[:], mds[b].rearrange("(q c) -> q c", q=P))
            scan = work.tile([P, T // P], _F32)
            nc.vector.tensor_tensor_scan(
                out=scan[:],
                data0=m_sb[:],
                data1=m_sb[:],
                initial=0.0,
                op0=AluOpType.max,
                op1=AluOpType.bypass,
            )
            # carry[q] = #{j: ends[j] < 256q}: compare ends against boundaries,
            # reduce over tokens (free dim by adds, partitions by matmul).
            bnd = work.tile([P, C * P], _F32)
            nc.gpsimd.iota(
                bnd[:],
                pattern=[[0, C], [T // P, P]],
                base=0,
                channel_multiplier=0,
                allow_small_or_imprecise_dtypes=True,
            )
            cmp = work.tile([P, C * P], _F32)
            nc.vector.tensor_tensor(
                out=cmp[:],
                in0=_view(ends_f, [[1, C], [0, P]]),
                in1=bnd[:],
                op=AluOpType.is_lt,
            )
            red = work.tile([P, P], _F32)
            nc.vector.tensor_tensor(
                out=red[:], in0=cmp[:, 0:P], in1=cmp[:, P : 2 * P], op=AluOpType.add
            )
            nc.vector.tensor_tensor(
                out=red[:], in0=red[:], in1=cmp[:, 2 * P : 3 * P], op=AluOpType.add
            )
            nc.vector.tensor_tensor(
                out=red[:], in0=red[:], in1=cmp[:, 3 * P : 4 * P], op=AluOpType.add
            )
            carry_ps = psum.tile([P, 1], _F32)
            nc.tensor.matmul(
                out=carry_ps[:], lhsT=red[:], rhs=ones_t[:], start=True, stop=True
            )
            idxf = work.tile([P, T // P], _F32)
            nc.vector.tensor_tensor(
                out=idxf[:],
                in0=scan[:],
                in1=carry_ps[:].to_broadcast([P, T // P]),
                op=AluOpType.max,
            )
            idx_i = work.tile([P, T // P], _I32)
            nc.vector.tensor_copy(idx_i[:], idxf[:])
            # 32 indirect gathers, one offset per partition (HW-verified
            # shape): gather g fills frame t = 32*p + g on partition p.
            big = work.tile([P, (T // P) * H], _F32)
            for g in range(T // P):
                nc.gpsimd.indirect_dma_start(
                    out=big[:, g * H : (g + 1) * H],
                    out_offset=None,
                    in_=encs[b][:, :],
                    in_offset=IndirectOffsetOnAxis(ap=idx_i[:, g : g + 1], axis=0),
                )
            ya = ys[b][:]
            nc.sync.dma_start(
                AP(ya.tensor, ya.offset, [[(T // P) * H, P], [H, T // P], [1, H]]), big[:]
            )
    nc.finalize()
    return nc


_PROGRAM = None


def _get_program() -> bass.Bass:
    global _PROGRAM
    if _PROGRAM is None:
        _PROGRAM = build_program()
    return _PROGRAM


def kernel(encoder_output, durations, max_length):
    from concourse.bass_utils import run_bass_kernel_spmd

    assert int(max_length) == T
    enc = np.asarray(encoder_output, dtype=np.float32).reshape(B, S, H)
    enc_ext = np.concatenate([enc, np.zeros((B, 1, H), np.float32)], axis=1)
    enc_ext = np.ascontiguousarray(enc_ext)
    dur = np.asarray(durations).astype(np.int32).reshape(B, S)
    dur_ext = np.concatenate([dur, np.ones((B, 1), np.int32)], axis=1)
    dur_ext = np.ascontiguousarray(dur_ext)

    nc = _get_program()
    in_maps = [
        {
            "dur": dur_ext[c * RPC : (c + 1) * RPC],
            **{
                f"enc{b}": np.ascontiguousarray(enc_ext[c * RPC + b])
                for b in range(RPC)
            },
        }
        for c in range(N_CORES)
    ]
    res = run_bass_kernel_spmd(nc, in_maps, list(range(N_CORES)))
    out = np.empty((B, T, H), dtype=np.float32)
    for c in range(N_CORES):
        for b in range(RPC):
            out[c * RPC + b] = res.results[c][f"y{b}"]
    return out
